# revision 38
# baseline (speedup 1.0000x reference)
"""PointerNet additive-attention + softmax kernel for 8 TRN2 NeuronCores.

Math (per batch b):
    ctx_t = ctx[b] @ W1i + b1i                  # [Te, H]
    dec_t = dec[b] @ W1h + b1h                  # [Td, H]
    scores[d, t] = w2 . tanh(dec_t[d] + ctx_t[t])   (+ b2, softmax-invariant)
    out[d] = softmax(scores[d, :])

Sharding: core c -> batch b = c//2, decoder rows d in [ (c%2)*256, +256 ).

Device layout: SBUF partition p = 32*hsub + dsub (hsub = h%4, dsub = d%32);
free axis of the tanh tiles = (hc = h//4 in [16], t in [512]).  ctx_t is
expanded to this layout by a host-replicated W1i (w1ix) so the projection
matmul lands it directly; dec_t is projected to [d-on-partition, h-free],
then shuffled into per-partition-scalar layout dec2x via 32 tiny
"selection" matmuls (slices of a 128x128 identity as lhsT, explicit
tile_position to hit psum partition quadrants).  Each 32-row d-block then
needs only 16 DVE tensor_scalar_adds + one big [128, 8192] bf16 tanh on
ACT + 16 accumulating PE matmuls with a host-built block w2 (w2x) that
drop scores as [d-on-partition, t-free] -- the layout softmax wants.
"""

import numpy as np
import ml_dtypes
from contextlib import ExitStack

import concourse.bass as bass
import concourse.bacc as bacc
import concourse.tile as tile
from concourse import mybir
from concourse.bass_utils import run_bass_kernel_spmd

B, Te, Td = 4, 512, 512
E, DE, H = 256, 512, 64
R = 256                 # decoder rows per core
NCORES = 8
NDBLK = 8               # 32-row d-blocks per core

FP32 = mybir.dt.float32
BF16 = mybir.dt.bfloat16
AF = mybir.ActivationFunctionType
AX = mybir.AxisListType
BF = ml_dtypes.bfloat16


def build_nc(npass: int = 1) -> bass.Bass:
    nc = bacc.Bacc("TRN2", target_bir_lowering=False, debug=False)

    ctxT_d = nc.declare_dram_parameter("ctxT", [128, 1024], BF16, isOutput=False)
    combo_d = nc.declare_dram_parameter("combo", [128, 1920], BF16, isOutput=False)
    w1ix_d = nc.declare_dram_parameter("w1ix", [128, 4096], BF16, isOutput=False)
    b2x_d = nc.declare_dram_parameter("b2x", [128, 128], FP32, isOutput=False)
    out_d = nc.declare_dram_parameter("out", [R, Te], FP32, isOutput=True)

    with tile.TileContext(nc) as tc:
        _body(tc, ctxT_d, combo_d, w1ix_d, b2x_d, out_d, npass=npass)
    return nc


def _body(tc, ctxT_d, combo_d, w1ix_d, b2x_d, out_d, npass=1):
    nc = tc.nc
    stack = ExitStack()
    ctxm = stack.enter_context
    const = ctxm(tc.tile_pool(name="const", bufs=1))

    ctxs = const.tile([128, 2 * Te], BF16, name="ctxs", tag="ctxs")
    combo = const.tile([128, 1920], BF16, name="combo", tag="combo")
    w1ix_s = const.tile([128, 4096], BF16, name="w1ix_s", tag="w1ix_s")
    b2x_s = const.tile([128, 128], FP32, name="b2x_s", tag="b2x_s")
    ctx2x = const.tile([128, 16 * Te], BF16, name="ctx2x", tag="ctx2x")
    dtn = const.tile([128, 128], BF16, name="dtn", tag="dtn")
    dec2x = const.tile([128, 128], FP32, name="dec2x", tag="dec2x")
    b2xv = const.tile([128, 128], FP32, name="b2xv", tag="b2xv")

    # bf16 consts travel in one packed DMA descriptor; slice views out of it
    decs = combo[:, 0:1024]
    w1h_s = combo[:, 1024:1280]
    eye_s = combo[:, 1280:1408]
    w2x_s = combo[:, 1408:1920]

    # ---- input DMAs: 4 descriptors over 3 queues --------------------------
    nc.scalar.dma_start(w1ix_s[:], w1ix_d[:])
    nc.gpsimd.dma_start(ctxs[:], ctxT_d[:])
    nc.sync.dma_start(combo[:], combo_d[:])
    nc.sync.dma_start(b2x_s[:], b2x_d[:])
    # prefetch b2x through DVE so the dec2x add below carries only a PE
    # wait: TensorTensor has a single hw sync-wait slot.
    nc.vector.tensor_copy(b2xv[:], b2x_s[:])

    pp = ctxm(tc.tile_pool(name="pp", bufs=3, space="PSUM"))
    dpp = ctxm(tc.tile_pool(name="dpp", bufs=2, space="PSUM"))
    dxp = ctxm(tc.tile_pool(name="dxp", bufs=1, space="PSUM"))
    scp = ctxm(tc.tile_pool(name="scp", bufs=2, space="PSUM"))
    sp = ctxm(tc.tile_pool(name="sp", bufs=3))
    tp = ctxm(tc.tile_pool(name="tp", bufs=3))
    ep = ctxm(tc.tile_pool(name="ep", bufs=2))
    op = ctxm(tc.tile_pool(name="op", bufs=2))
    redp = ctxm(tc.tile_pool(name="redp", bufs=1))

    for _pass in range(npass):
        # ---- dec projection: dec_tN[c][dp, h'] = dec[128c+dp] @ W1hp ------
        for c in range(2):
            dps = dpp.tile([128, H], FP32, name="dps")
            for k in range(4):
                nc.tensor.matmul(dps[:],
                                 decs[:, (k * 2 + c) * 128:(k * 2 + c + 1) * 128],
                                 w1h_s[:, k * H:(k + 1) * H],
                                 start=(k == 0), stop=(k == 3))
            nc.vector.tensor_copy(dtn[:, c * H:(c + 1) * H], dps[:])

        # ---- shuffle dec_t into per-partition-scalar layout dec2x ---------
        # dec2x[32*hsub+dsub, 16*dblk+hc] = dec_t[32*dblk+dsub, 4*hc+hsub]
        dxps = dxp.tile([128, 128], FP32, name="dxps")
        for c in range(2):
            for dbc in range(4):
                for hsub in range(4):
                    f0 = (4 * c + dbc) * 16
                    nc.tensor.matmul(
                        dxps[32 * hsub:32 * hsub + 32, f0:f0 + 16],
                        eye_s[:, 32 * dbc:32 * dbc + 32],
                        dtn[:, c * H + 16 * hsub: c * H + 16 * hsub + 16],
                        start=True, stop=True, tile_position=(0, 32 * hsub))
        # fold (b1i + b1h) in while evacuating (b2x is all-zero for spec)
        nc.vector.tensor_add(dec2x[:], dxps[:], b2xv[:])

        # ---- ctx projection into expanded layout --------------------------
        # ctx2x[32*hsub+dsub, hc*512+t] = ctx_t[t, 4*hc+hsub]
        # Emitted per-hc inside the dblk0 group loop below so the first tanh
        # chunk is not queued behind all 16 projections/evacuations.
        def ctx_proj(hc):
            cps = pp.tile([128, Te], FP32, name="cps")
            for ec in range(2):
                nc.tensor.matmul(
                    cps[:],
                    w1ix_s[:, (ec * 16 + hc) * 128:(ec * 16 + hc + 1) * 128],
                    ctxs[:, ec * Te:(ec + 1) * Te],
                    start=(ec == 0), stop=(ec == 1))
            # GPSIMD cannot read PSUM on real hw: split evacs over ACT + DVE
            if hc % 2 == 0:
                nc.scalar.copy(ctx2x[:, hc * Te:(hc + 1) * Te], cps[:])
            else:
                nc.vector.tensor_copy(ctx2x[:, hc * Te:(hc + 1) * Te], cps[:])

        scs = [None, None]
        exs = [None, None]

        def softmax_a(r):
            # |scores| <= ~8 for this problem, so skip the max-subtraction:
            # exp is safe in fp32 and the reduce_max dependency disappears.
            exs[r] = ep.tile([128, Te], FP32, name=f"ex{r}", tag="ex")
            nc.scalar.activation(exs[r][:], scs[r][:], AF.Exp)

        def softmax_b(r):
            ssum = redp.tile([128, 1], FP32, name=f"ssum{r}")
            nc.vector.reduce_sum(ssum[:], exs[r][:], axis=AX.X)
            rec = redp.tile([128, 1], FP32, name=f"rec{r}")
            nc.vector.reciprocal(rec[:], ssum[:])
            o = op.tile([128, Te], FP32, name=f"o{r}", tag="o")
            nc.vector.tensor_scalar_mul(o[:], exs[r][:], rec[:, 0:1])
            nc.sync.dma_start(out_d[r * 128:(r + 1) * 128, :], o[:])

        # ---- main loop: 8 d-blocks of 32 rows -----------------------------
        # dblk0 tanh is chunked so ACT starts as soon as the first hc-group
        # of ctx2x lands; dblk7 is chunked so the final matmul burst (and
        # thus softmax of round 1) starts before the whole tanh finishes.
        for dblk in range(NDBLK):
            r, blk = dblk // 4, dblk % 4
            if blk == 0:
                scs[r] = scp.tile([128, Te], FP32, name=f"sc{r}", tag="sc")
            S = sp.tile([128, 16 * Te], BF16, name="S")
            T = tp.tile([128, 16 * Te], BF16, name="T")
            if dblk == 0:
                groups = [(0, 4), (4, 8), (8, 12), (12, 16)]
            elif dblk >= 5:
                groups = [(0, 8), (8, 12), (12, 16)]
            else:
                groups = [(0, 16)]
            for h0, h1 in groups:
                if dblk == 0:
                    for hc in range(h0, h1):
                        ctx_proj(hc)
                for hc in range(h0, h1):
                    nc.vector.tensor_scalar_add(
                        S[:, hc * Te:(hc + 1) * Te],
                        ctx2x[:, hc * Te:(hc + 1) * Te],
                        dec2x[:, dblk * 16 + hc: dblk * 16 + hc + 1])
                nc.scalar.activation(T[:, h0 * Te:h1 * Te],
                                     S[:, h0 * Te:h1 * Te], AF.Tanh)
                for hc in range(h0, h1):
                    nc.tensor.matmul(scs[r][32 * blk:32 * blk + 32, :],
                                     w2x_s[:, hc * 32:(hc + 1) * 32],
                                     T[:, hc * Te:(hc + 1) * Te],
                                     start=(hc == 0), stop=(hc == 15),
                                     tile_position=(0, 32 * blk))
            if dblk == 4:
                softmax_a(0)
            if dblk == 5:
                softmax_b(0)
        softmax_a(1)
        softmax_b(1)
    stack.close()


_NC_CACHE = None


def _get_nc():
    global _NC_CACHE
    if _NC_CACHE is None:
        _NC_CACHE = build_nc()
        _NC_CACHE.finalize()
    return _NC_CACHE


def make_in_maps(ctx, decoder_states, W1i, b1i, W1h, b1h, w2, b2=None):
    ctx = np.asarray(ctx, np.float32)              # [B, Te, E]
    dec = np.asarray(decoder_states, np.float32)   # [B, Td, DE]
    W1i = np.asarray(W1i, np.float32)              # [E, H]
    W1h = np.asarray(W1h, np.float32)              # [DE, H]
    w2 = np.asarray(w2, np.float32).reshape(H)
    bb = np.zeros(H, np.float32)
    if b1i is not None:
        bb = bb + np.asarray(b1i, np.float32).reshape(H)
    if b1h is not None:
        bb = bb + np.asarray(b1h, np.float32).reshape(H)

    p = np.arange(128)
    hcs = np.arange(16)
    hmap = 4 * hcs[:, None] + p[None, :] // 32            # [16, 128]
    w1ix = W1i.reshape(2, 128, H)[:, :, hmap]             # [2, 128, 16, 128]
    w1ix = np.ascontiguousarray(
        w1ix.reshape(2, 128, 2048).transpose(1, 0, 2).reshape(128, 4096)
    ).astype(BF)

    hp = np.arange(H)
    w1hp = W1h.reshape(4, 128, H)[:, :, 4 * (hp % 16) + hp // 16]
    w1hp = w1hp.transpose(1, 0, 2).reshape(128, 4 * H)

    w2x = np.zeros((128, 512), np.float32)
    m = p % 32
    for hc in range(16):
        w2x[p, hc * 32 + m] = w2[4 * hc + p // 32]

    eye = np.eye(128, dtype=np.float32)

    f = np.arange(128)
    b2x = bb[4 * (f[None, :] % 16) + p[:, None] // 32].astype(np.float32)

    in_maps = []
    for c in range(NCORES):
        b, half = c // 2, c % 2
        ctxT = np.ascontiguousarray(
            ctx[b].T.reshape(2, 128, Te).transpose(1, 0, 2).reshape(128, 1024)
        ).astype(BF)
        decsh = dec[b, half * R:(half + 1) * R, :]        # [R, DE]
        dT = decsh.T.reshape(4, 128, 2, 128)              # [k, de, c, dp]
        decTx = (dT.transpose(0, 2, 1, 3).reshape(8, 128, 128)
                 .transpose(1, 0, 2).reshape(128, 1024))
        combo = np.ascontiguousarray(
            np.concatenate([decTx, w1hp, eye, w2x], axis=1)).astype(BF)
        in_maps.append({
            "ctxT": ctxT, "combo": combo, "w1ix": w1ix, "b2x": b2x,
        })
    return in_maps


def gather(results) -> np.ndarray:
    out = np.empty((B, Td, Te), np.float32)
    for c in range(NCORES):
        b, half = c // 2, c % 2
        out[b, half * R:(half + 1) * R, :] = results[c]["out"]
    return out


def kernel(**inputs) -> np.ndarray:
    nc = _get_nc()
    in_maps = make_in_maps(**inputs)
    res = run_bass_kernel_spmd(nc, in_maps, list(range(NCORES)))
    return gather(res.results)
